# revision 60
# baseline (speedup 1.0000x reference)
"""Trainium2 Bass kernel for multi-head self-attention with RoPE.

Problem shapes (hardcoded): x [2, 2048, 1024], 16 heads x 64 dim, fp32.
Sharding: (batch x head-group) -- core c owns batch c//4 and the 4 heads
[4*(c%4), 4*(c%4)+4), i.e. 256 local head dims handled as 2 "pairs" of
128 partition dims.  Host sums the 4 partial outputs per batch and adds
the constant row bv @ Wo + bo (exact: attention rows sum to 1, so the
V-bias contribution to the output is position independent).

All matmul operands are bf16 (tolerance is 2e-2); PSUM accumulation is
fp32.  Layout notes:
 - xT [D, T] in SBUF serves both as moving operand (Q/K projections,
   contracted dim D on partitions) and stationary operand (V natural
   projection: lhsT = xT chunk gives V in [t, dl] layout directly, no
   PE transposes).
 - RoPE uses the interleaved-pair column permutation of Wq/Wk so the
   rotation partner of partition p is p^1 (stream_shuffle within-32).
 - Scores are computed transposed (S^T = K Q^T, [k, q]); softmax
   denominators come from a ones-column in the V tiles; exp is split
   between the Activation engine (true Exp) and the DVE (fast-exp bit
   trick writing bf16 bits as int16).
 - Out projection contracts the 256 local dims as 2 chained matmuls on
   normalized ctx^T tiles; partial outputs stored bf16.
"""

import os
import numpy as np
from ml_dtypes import bfloat16

# defensive: flaky device state has produced transient NaN outputs that a
# core reset clears; request reset and retry on non-finite results
os.environ.setdefault("NEURON_RT_RESET_CORES", "1")

import concourse.bass as bass
import concourse.tile as tile
from concourse import mybir
from concourse.bass_utils import run_bass_kernel_spmd

N_CORES = 8
B, T, D = 2, 2048, 1024
H, HD = 16, 64            # total heads, head dim
HL = 4                    # heads per core
DL = HL * HD              # local head dims (256)
NP = 2                    # pairs of 128 partition dims per core
CC = D // 128             # contraction chunks (8)
NT = T // 512             # 512-wide t-chunks (4)
NKT = T // 128            # 128-row k-tiles (16)
F32 = mybir.dt.float32
BF16 = mybir.dt.bfloat16
I16 = mybir.dt.int16

# exp column split: scalar engine does CS cols of each [128,1024] tile,
# DVE does the remaining CV via the bit-trick fast exp.
CS = 1024
CV = 1024 - CS
# fast-exp constants: bf16_bits(exp(0.125*s)) ~= round(s*K1 + K2)
K1 = 128.0 * 0.125 * 1.4426950408889634
K2 = 128.0 * (127.0 - 0.04305)

# within-32 adjacent-pair swap for RoPE (partition p <-> p^1)
SWAP_MASK = [i ^ 1 for i in range(32)]

_CACHE = {}
LAST_RESULT = None


def _build_nc(dbg_names=()):
    from concourse import bacc
    nc = bacc.Bacc("TRN2", target_bir_lowering=False, debug=False,
                   num_devices=N_CORES)
    xt = nc.dram_tensor("xt", [D, T], BF16, kind="ExternalInput").ap()
    wq = nc.dram_tensor("wq", [D, DL], BF16, kind="ExternalInput").ap()
    wk = nc.dram_tensor("wk", [D, DL], BF16, kind="ExternalInput").ap()
    wv = nc.dram_tensor("wv", [D, DL], BF16, kind="ExternalInput").ap()
    wo = nc.dram_tensor("wo", [DL, D], BF16, kind="ExternalInput").ap()
    cosb = nc.dram_tensor("cosb", [128, T], BF16, kind="ExternalInput").ap()
    sinb = nc.dram_tensor("sinb", [128, T], BF16, kind="ExternalInput").ap()
    # columns: bq pair0, bq pair1, bk pair0, bk pair1 (interleaved layout)
    bqk = nc.dram_tensor("bqk", [128, 4], F32, kind="ExternalInput").ap()
    out = nc.dram_tensor("out", [T, D], BF16, kind="ExternalOutput").ap()

    dbg = {}
    dbg_shapes = {
        "dbg_qrot": ([128, T], BF16), "dbg_krot": ([128, T], BF16),
        "dbg_vh": ([128, NKT * HL * (HD + 1)], BF16),
        "dbg_pt": ([128, 1024], BF16), "dbg_cx": ([HD + 1, 512], F32),
        "dbg_stk": ([128, 512], BF16),
    }
    for n in dbg_names:
        shp, dt_ = dbg_shapes[n]
        dbg[n] = nc.dram_tensor(n, shp, dt_, kind="ExternalOutput").ap()

    with tile.TileContext(nc) as tc:
        _body(tc, xt, wq, wk, wv, wo, cosb, sinb, bqk, out, dbg)

    nc.compile()
    return nc


def _body(tc, xt, wq, wk, wv, wo, cosb, sinb, bqk, out, dbg={}):
    nc = tc.nc
    from contextlib import ExitStack
    with ExitStack() as ctx:
        consts = ctx.enter_context(tc.tile_pool(name="consts", bufs=1))
        work = ctx.enter_context(tc.tile_pool(name="work", bufs=2))
        work3 = ctx.enter_context(tc.tile_pool(name="work3", bufs=4))
        # single 3-deep [128,1024] ring (6 banks) shared by scores, proj
        # chains and out-proj tiles + 2 banks for the ctx accumulators
        ps_big = ctx.enter_context(
            tc.tile_pool(name="psb", bufs=3, space="PSUM"))
        ps_cx = ctx.enter_context(
            tc.tile_pool(name="psc", bufs=2, space="PSUM"))

        # ---- constants / persistent SBUF ----
        # DMA order matters: the first Q projection chain needs wq + the
        # first 512-col block of every xt chunk, so those go first and the
        # PE can start ~4us in instead of waiting for the full load.
        # few big DMAs: per-DMA issue overhead (~0.6us) dominates the
        # startup when the first chain waits on many small transfers
        wq_sb = consts.tile([128, CC * DL], BF16)
        # xt per (tc, ci-half): [128, 4*512] tiles
        xt_tc = [consts.tile([128, 4 * 512], BF16, name=f"xtt{tcn}_{h}")
                 for tcn in range(NT) for h in range(2)]

        def xt_ap(ci, tcn, lo, width):
            return xt_tc[tcn * 2 + ci // 4][
                :, (ci % 4) * 512 + lo:(ci % 4) * 512 + lo + width]

        def load_xt(tcn, nsplit=1):
            ts0 = tcn * 512
            for h in range(2):
                for j in range(nsplit):
                    w = 512 // nsplit
                    nc.sync.dma_start(
                        xt_tc[tcn * 2 + h][:, :].rearrange(
                            "p (cc t) -> p cc t", cc=4)[:, :, j * w:(j + 1) * w],
                        xt[h * 512:(h + 1) * 512,
                           ts0 + j * w:ts0 + (j + 1) * w].rearrange(
                            "(cc p) t -> p cc t", p=128))

        for h in range(2):
            nc.sync.dma_start(
                wq_sb[:, h * 4 * DL:(h + 1) * 4 * DL].rearrange(
                    "p (cc m) -> p cc m", cc=4),
                wq[h * 512:(h + 1) * 512, :].rearrange(
                    "(cc p) m -> p cc m", p=128))
        load_xt(0)
        wk_sb = consts.tile([128, CC * DL], BF16)
        nc.sync.dma_start(
            wk_sb[:, :].rearrange("p (cc m) -> p cc m", cc=CC),
            wk.rearrange("(cc p) m -> p cc m", p=128))
        bqk_sb = consts.tile([128, 4], F32)
        nc.sync.dma_start(bqk_sb[:, :], bqk)
        cos_sb = consts.tile([128, T], BF16)
        nc.sync.dma_start(cos_sb[:, :], cosb)
        sin_sb = consts.tile([128, T], BF16)
        nc.sync.dma_start(sin_sb[:, :], sinb)
        for tcn in range(1, NT):
            load_xt(tcn)
        wv_sb = consts.tile([128, CC * DL], BF16)
        nc.sync.dma_start(
            wv_sb[:, :].rearrange("p (cc m) -> p cc m", cc=CC),
            wv.rearrange("(cc p) m -> p cc m", p=128))
        wo_sb = consts.tile([128, NP * D], BF16)
        nc.sync.dma_start(
            wo_sb[:, :].rearrange("p (pr d) -> p pr d", pr=NP),
            wo.rearrange("(pr p) d -> p pr d", p=128))

        qrot = [consts.tile([128, T], BF16, name=f"qrot{p}")
                for p in range(NP)]
        krot = [consts.tile([128, T], BF16, name=f"krot{p}")
                for p in range(NP)]
        # V tiles: per k-tile, HL blocks of (64 v-dims + ones column)
        vh = consts.tile([128, NKT * HL * (HD + 1)], BF16)
        nc.gpsimd.memset(
            vh[:, :].rearrange("p (g c) -> p g c", c=HD + 1)[:, :, HD:HD + 1],
            1.0)

        # ======== Q/K projections + RoPE ========
        def qk_chain(name, pair, tcn):
            wsb = wq_sb if name == "q" else wk_sb
            bcol = 0 if name == "q" else 2
            dst = qrot[pair] if name == "q" else krot[pair]
            ts = slice(tcn * 512, (tcn + 1) * 512)
            pp = ps_big.tile([128, 1024], F32, tag="sp",
                            name=f"pp_{name}_{pair}_{tcn}")
            for ci in range(CC):
                nc.tensor.matmul(
                    pp[:, 0:512],
                    wsb[:, ci * DL + pair * 128:ci * DL + (pair + 1) * 128],
                    xt_ap(ci, tcn, 0, 512),
                    start=(ci == 0), stop=(ci == CC - 1))
            xb = work.tile([128, 512], BF16, tag="xb")
            nc.scalar.activation(
                xb[:, :], pp[:, 0:512],
                mybir.ActivationFunctionType.Identity,
                bias=bqk_sb[:, bcol + pair:bcol + pair + 1], scale=1.0)
            sh = work.tile([128, 512], BF16, tag="sh")
            nc.vector.stream_shuffle(sh[:, :], xb[:, :], SWAP_MASK)
            m1 = work.tile([128, 512], BF16, tag="m1")
            nc.vector.tensor_mul(m1[:, :], xb[:, :], cos_sb[:, ts])
            m2 = work.tile([128, 512], BF16, tag="m2")
            nc.vector.tensor_mul(m2[:, :], sh[:, :], sin_sb[:, ts])
            nc.vector.tensor_add(dst[:, ts], m1[:, :], m2[:, :])

        # tc-outer so each new 512-col block of xt serves the chains before
        # the next block is needed; pair1's chains are deferred into the
        # first attention block, where the scalar engine (attention pacer)
        # has idle capacity to overlap them
        for tcn in range(NT):
            for name in ("q", "k"):
                qk_chain(name, 0, tcn)
        pair1_chains = [(name, tcn) for tcn in range(NT)
                        for name in ("q", "k")]

        # ======== V natural projection ========
        for tt in range(NKT):
            vp = ps_big.tile([128, 1024], F32, tag="sp", name=f"vp_{tt}")
            to = (tt % 4) * 128
            for ci in range(CC):
                nc.tensor.matmul(
                    vp[:, 0:DL],
                    xt_ap(ci, tt // 4, to, 128),
                    wv_sb[:, ci * DL:(ci + 1) * DL],
                    start=(ci == 0), stop=(ci == CC - 1))
            dst3 = vh[:, tt * HL * (HD + 1):(tt + 1) * HL * (HD + 1)]
            nc.scalar.activation(
                dst3.rearrange("p (h c) -> p h c", c=HD + 1)[:, :, 0:HD],
                vp[:, 0:DL].rearrange("p (h c) -> p h c", c=HD),
                mybir.ActivationFunctionType.Copy)

        if "dbg_qrot" in dbg:
            nc.sync.dma_start(dbg["dbg_qrot"], qrot[0][:, :])
        if "dbg_krot" in dbg:
            nc.sync.dma_start(dbg["dbg_krot"], krot[0][:, :])
        if "dbg_vh" in dbg:
            nc.sync.dma_start(dbg["dbg_vh"], vh[:, :])

        # ======== attention (per 512-wide q chunk) + out projection ====
        # The out projection of chunk qc-1 is software-pipelined into the
        # kt loop of chunk qc so the in-order PE never stalls waiting for
        # the normalize chain (reciprocal -> broadcast -> mul -> dma).
        stk_tiles = [[consts.tile([128, 512], BF16, name=f"stk{i}_{p}")
                      for p in range(NP)] for i in range(2)]

        def out_proj_tsub(qc, tsub, drain=False):
            # op lives in the proj-phase "pp" ring (idle during attention)
            # so score matmuls never stall behind osb copies.
            stk = stk_tiles[qc % 2]
            row0 = qc * 512 + tsub * 128
            osb = work.tile([128, D], BF16, tag="osb")
            op = ps_big.tile([128, 1024], F32, tag="sp",
                             name=f"op_{qc}_{tsub}")
            for dc in range(2):
                for pair in range(NP):
                    nc.tensor.matmul(
                        op[:, dc * 512:(dc + 1) * 512],
                        stk[pair][:, tsub * 128:(tsub + 1) * 128],
                        wo_sb[:, pair * D + dc * 512:
                              pair * D + (dc + 1) * 512],
                        start=(pair == 0), stop=(pair == NP - 1))
            if drain:
                nc.scalar.copy(osb[:, 0:512], op[:, 0:512])
                nc.vector.tensor_copy(osb[:, 512:1024], op[:, 512:1024])
            else:
                nc.vector.tensor_copy(osb[:, :], op[:, :])
            nc.sync.dma_start(out[row0:row0 + 128, :], osb[:, :])

        for qc in range(NT):
            qs = slice(qc * 512, (qc + 1) * 512)
            stk = stk_tiles[qc % 2]
            for pair in range(NP):
                cx = [ps_cx.tile([HD + 1, 512], F32, tag="cx",
                                 name=f"cx_{qc}_{pair}_{h}")
                      for h in range(2)]
                pts = {}

                def emit_pv(kt):
                    for h in range(2):
                        g = (pair * 2 + h)
                        base = kt * HL * (HD + 1) + g * (HD + 1)
                        nc.tensor.matmul(
                            cx[h][:, :],
                            vh[:, base:base + HD + 1],
                            pts[kt][:, h * 512:(h + 1) * 512],
                            start=(kt == 0), stop=(kt == NKT - 1))
                    del pts[kt]

                for kt in range(NKT):
                    sp = ps_big.tile([128, 1024], F32, tag="sp",
                                     name=f"sp_{qc}_{pair}_{kt}")
                    for h in range(2):
                        hs = slice(h * HD, (h + 1) * HD)
                        nc.tensor.matmul(
                            sp[:, h * 512:(h + 1) * 512],
                            krot[pair][hs, kt * 128:(kt + 1) * 128],
                            qrot[pair][hs, qs], start=True, stop=True)
                    pt = work3.tile([128, 1024], BF16, tag="pt")
                    pts[kt] = pt
                    nc.scalar.activation(
                        pt[:, :], sp[:, :],
                        mybir.ActivationFunctionType.Exp, scale=0.125)
                    if qc == 0 and pair == 0 and kt == 0 and "dbg_pt" in dbg:
                        tpt = work.tile([128, 1024], BF16, tag="tpt", bufs=1)
                        nc.vector.tensor_copy(tpt[:, :], pt[:, :])
                        nc.sync.dma_start(dbg["dbg_pt"], tpt[:, :])
                    # PV runs two kt behind S: covers both the exp latency
                    # (pt[kt] ready ~1.4us after S[kt]) and the first PV's
                    # wait on the cx ring (previous pair's normalize)
                    if kt >= 2:
                        emit_pv(kt - 2)
                    # deferred pair1 projections overlap the first block
                    if qc == 0 and pair == 0 and kt % 2 == 0:
                        nm, tn = pair1_chains[kt // 2]
                        qk_chain(nm, 1, tn)
                    # pipelined out projection of the previous q chunk
                    # (2 tsubs in each pair's kt loop, spread out)
                    if qc > 0 and kt % 8 == 5:
                        out_proj_tsub(qc - 1, pair * 2 + kt // 8)
                emit_pv(NKT - 2)
                emit_pv(NKT - 1)

                if qc == 0 and pair == 0 and "dbg_cx" in dbg:
                    tcx = work.tile([HD + 1, 512], F32, tag="tcx", bufs=1)
                    nc.vector.tensor_copy(tcx[:, :], cx[0][:, :])
                    nc.sync.dma_start(dbg["dbg_cx"], tcx[:, :])
                # Drain cx PSUM fast with plain copies (frees the ring for
                # the next pair), then normalize from the SBUF copies.  On
                # the final pair nothing reuses the ring, so normalize
                # straight from PSUM (one hop less before the drain).
                last = (qc == NT - 1)
                stk_p = stk[pair]
                ctxsb = []
                den64s = []
                for h in range(2):
                    if last:
                        ctxsb.append(None)
                    else:
                        csb = work.tile([HD, 512], F32, tag=f"ctxsb{h}")
                        nc.vector.tensor_copy(csb[:, :], cx[h][0:HD, :])
                        ctxsb.append(csb)
                    den64 = work.tile([HD + 1, 512], F32, tag="den64")
                    nc.vector.tensor_copy(den64[HD:HD + 1, :],
                                          cx[h][HD:HD + 1, :])
                    den64s.append(den64)
                for h in range(2):
                    den0 = work.tile([1, 512], F32, tag="den0")
                    nc.sync.dma_start(den0[0:1, :], den64s[h][HD:HD + 1, :])
                    rcp0 = work.tile([1, 512], F32, tag="rcp0")
                    nc.vector.reciprocal_approx_fast(rcp0[0:1, :],
                                                     den0[0:1, :])
                    bc = work.tile([HD, 512], F32, tag="bc")
                    nc.gpsimd.partition_broadcast(bc[:, :], rcp0[0:1, :],
                                                  channels=HD)
                    src = cx[h][0:HD, :] if last else ctxsb[h][:, :]
                    if h == 0:
                        nc.vector.tensor_mul(stk_p[0:HD, :], src, bc[:, :])
                    else:
                        cn1 = work.tile([HD, 512], BF16, tag="cn1")
                        nc.vector.tensor_mul(cn1[:, :], src, bc[:, :])
                        nc.sync.dma_start(stk_p[HD:128, :], cn1[:, :])

            if qc == 0 and "dbg_stk" in dbg:
                nc.sync.dma_start(dbg["dbg_stk"], stk[0][:, :])

        # drain: out projection of the final q chunk.  The pair0 chain
        # pieces depend only on stk[pair0] (ready since mid-chunk), so
        # open as many accumulation groups as the ring allows before the
        # pair1 normalize lands.
        qcl = NT - 1
        stk = stk_tiles[qcl % 2]

        def drain_piece(op, tsub, pair):
            for dc in range(2):
                nc.tensor.matmul(
                    op[:, dc * 512:(dc + 1) * 512],
                    stk[pair][:, tsub * 128:(tsub + 1) * 128],
                    wo_sb[:, pair * D + dc * 512:pair * D + (dc + 1) * 512],
                    start=(pair == 0), stop=(pair == NP - 1))

        def drain_close(op, tsub):
            drain_piece(op, tsub, 1)
            row0 = qcl * 512 + tsub * 128
            osb = work.tile([128, D], BF16, tag="osb")
            nc.scalar.copy(osb[:, 0:512], op[:, 0:512])
            nc.vector.tensor_copy(osb[:, 512:1024], op[:, 512:1024])
            nc.sync.dma_start(out[row0:row0 + 128, :], osb[:, :])

        dops = [ps_big.tile([128, 1024], F32, tag="sp", name=f"dop_{t}")
                for t in range(3)]
        for t in range(3):
            drain_piece(dops[t], t, 0)
        drain_close(dops[0], 0)
        op3 = ps_big.tile([128, 1024], F32, tag="sp", name="dop_3")
        drain_piece(op3, 3, 0)
        drain_close(dops[1], 1)
        drain_close(dops[2], 2)
        drain_close(op3, 3)


def _rope_tables():
    """cos/sin tables in the interleaved-pair partition layout (bf16)."""
    pos = np.arange(T, dtype=np.float32)[:, None]                 # [T, 1]
    freq_seq = np.arange(HD // 2, dtype=np.float32)
    inv_freq = (1.0 / (10000.0 ** (freq_seq / np.float32(HD // 2)))).astype(
        np.float32)
    ang = pos * inv_freq[None, :]                                 # [T, 32]
    sin = np.sin(ang).astype(np.float32)                          # [T, 32]
    cos = np.cos(ang).astype(np.float32)
    cosb = np.empty((128, T), dtype=np.float32)
    sinb = np.empty((128, T), dtype=np.float32)
    for p in range(128):
        r = p % HD
        j = r // 2
        second = r % 2
        cosb[p] = cos[:, j]
        sinb[p] = sin[:, j] if second else -sin[:, j]
    return cosb.astype(bfloat16), sinb.astype(bfloat16)


def _perm():
    """interleaved-pair permutation of each head's 64 dims:
    new[h*64 + 2j] = old[h*64 + j]; new[h*64 + 2j + 1] = old[h*64 + 32 + j]"""
    p = np.arange(DL)
    return (p // HD) * HD + (p % HD) // 2 + (p % 2) * (HD // 2)


def kernel(**inputs):
    global LAST_RESULT
    x = np.asarray(inputs["x"], dtype=np.float32)
    Wq = np.asarray(inputs["Wq"], dtype=np.float32)
    Wk = np.asarray(inputs["Wk"], dtype=np.float32)
    Wv = np.asarray(inputs["Wv"], dtype=np.float32)
    Wo = np.asarray(inputs["Wo"], dtype=np.float32)
    bq = np.asarray(inputs["bq"], dtype=np.float32)
    bk = np.asarray(inputs["bk"], dtype=np.float32)
    bv = np.asarray(inputs["bv"], dtype=np.float32)
    bo = np.asarray(inputs["bo"], dtype=np.float32)

    if "nc" not in _CACHE:
        _CACHE["nc"] = _build_nc()
    nc = _CACHE["nc"]

    xT = np.ascontiguousarray(x.transpose(0, 2, 1)).astype(bfloat16)
    cosb, sinb = _rope_tables()
    perm = _perm()

    in_maps = []
    for c in range(N_CORES):
        b, hg = c // 4, c % 4
        cs = slice(hg * DL, (hg + 1) * DL)
        bq_c = bq[cs][perm]
        bk_c = bk[cs][perm]
        bqk_c = np.stack([bq_c[0:128], bq_c[128:256],
                          bk_c[0:128], bk_c[128:256]], axis=1)
        in_maps.append({
            "xt": xT[b],
            "wq": np.ascontiguousarray(Wq[:, cs][:, perm]).astype(bfloat16),
            "wk": np.ascontiguousarray(Wk[:, cs][:, perm]).astype(bfloat16),
            "wv": np.ascontiguousarray(Wv[:, cs]).astype(bfloat16),
            "wo": np.ascontiguousarray(Wo[cs, :]).astype(bfloat16),
            "cosb": cosb, "sinb": sinb,
            "bqk": np.ascontiguousarray(bqk_c).astype(np.float32),
        })

    trace = bool(int(os.environ.get("BASS_KERNEL_TRACE", "0")))
    for attempt in range(3):
        res = run_bass_kernel_spmd(nc, in_maps,
                                   core_ids=list(range(N_CORES)),
                                   trace=trace)
        LAST_RESULT = res
        acc = np.zeros((B, T, D), dtype=np.float32)
        for c in range(N_CORES):
            acc[c // 4] += res.results[c]["out"].astype(np.float32)
        if np.isfinite(acc).all():
            break
    const_row = (bv.astype(np.float64) @ Wo.astype(np.float64)
                 + bo.astype(np.float64)).astype(np.float32)
    return acc + const_row[None, None, :]


# revision 61
# speedup vs baseline: 1.0072x; 1.0072x over previous
"""Trainium2 Bass kernel for multi-head self-attention with RoPE.

Problem shapes (hardcoded): x [2, 2048, 1024], 16 heads x 64 dim, fp32.
Sharding: (batch x head-group) -- core c owns batch c//4 and the 4 heads
[4*(c%4), 4*(c%4)+4), i.e. 256 local head dims handled as 2 "pairs" of
128 partition dims.  Host sums the 4 partial outputs per batch and adds
the constant row bv @ Wo + bo (exact: attention rows sum to 1, so the
V-bias contribution to the output is position independent).

All matmul operands are bf16 (tolerance is 2e-2); PSUM accumulation is
fp32.  Layout notes:
 - xT [D, T] in SBUF serves both as moving operand (Q/K projections,
   contracted dim D on partitions) and stationary operand (V natural
   projection: lhsT = xT chunk gives V in [t, dl] layout directly, no
   PE transposes).
 - RoPE uses the interleaved-pair column permutation of Wq/Wk so the
   rotation partner of partition p is p^1 (stream_shuffle within-32).
 - Scores are computed transposed (S^T = K Q^T, [k, q]); softmax
   denominators come from a ones-column in the V tiles; exp is split
   between the Activation engine (true Exp) and the DVE (fast-exp bit
   trick writing bf16 bits as int16).
 - Out projection contracts the 256 local dims as 2 chained matmuls on
   normalized ctx^T tiles; partial outputs stored bf16.
"""

import os
import numpy as np
from ml_dtypes import bfloat16

# defensive: flaky device state has produced transient NaN outputs that a
# core reset clears; request reset and retry on non-finite results
os.environ.setdefault("NEURON_RT_RESET_CORES", "1")

import concourse.bass as bass
import concourse.tile as tile
from concourse import mybir
from concourse.bass_utils import run_bass_kernel_spmd

N_CORES = 8
B, T, D = 2, 2048, 1024
H, HD = 16, 64            # total heads, head dim
HL = 4                    # heads per core
DL = HL * HD              # local head dims (256)
NP = 2                    # pairs of 128 partition dims per core
CC = D // 128             # contraction chunks (8)
NT = T // 512             # 512-wide t-chunks (4)
NKT = T // 128            # 128-row k-tiles (16)
F32 = mybir.dt.float32
BF16 = mybir.dt.bfloat16
I16 = mybir.dt.int16

# exp column split: scalar engine does CS cols of each [128,1024] tile,
# DVE does the remaining CV via the bit-trick fast exp.
CS = 1024
CV = 1024 - CS
# fast-exp constants: bf16_bits(exp(0.125*s)) ~= round(s*K1 + K2)
K1 = 128.0 * 0.125 * 1.4426950408889634
K2 = 128.0 * (127.0 - 0.04305)

# within-32 adjacent-pair swap for RoPE (partition p <-> p^1)
SWAP_MASK = [i ^ 1 for i in range(32)]

_CACHE = {}
LAST_RESULT = None


def _build_nc(dbg_names=()):
    from concourse import bacc
    nc = bacc.Bacc("TRN2", target_bir_lowering=False, debug=False,
                   num_devices=N_CORES)
    xt = nc.dram_tensor("xt", [D, T], BF16, kind="ExternalInput").ap()
    wq = nc.dram_tensor("wq", [D, DL], BF16, kind="ExternalInput").ap()
    wk = nc.dram_tensor("wk", [D, DL], BF16, kind="ExternalInput").ap()
    wv = nc.dram_tensor("wv", [D, DL], BF16, kind="ExternalInput").ap()
    wo = nc.dram_tensor("wo", [DL, D], BF16, kind="ExternalInput").ap()
    cosb = nc.dram_tensor("cosb", [128, T], BF16, kind="ExternalInput").ap()
    sinb = nc.dram_tensor("sinb", [128, T], BF16, kind="ExternalInput").ap()
    # columns: bq pair0, bq pair1, bk pair0, bk pair1 (interleaved layout)
    bqk = nc.dram_tensor("bqk", [128, 4], F32, kind="ExternalInput").ap()
    out = nc.dram_tensor("out", [T, D], BF16, kind="ExternalOutput").ap()

    dbg = {}
    dbg_shapes = {
        "dbg_qrot": ([128, T], BF16), "dbg_krot": ([128, T], BF16),
        "dbg_vh": ([128, NKT * HL * (HD + 1)], BF16),
        "dbg_pt": ([128, 1024], BF16), "dbg_cx": ([HD + 1, 512], F32),
        "dbg_stk": ([128, 512], BF16),
    }
    for n in dbg_names:
        shp, dt_ = dbg_shapes[n]
        dbg[n] = nc.dram_tensor(n, shp, dt_, kind="ExternalOutput").ap()

    with tile.TileContext(nc) as tc:
        _body(tc, xt, wq, wk, wv, wo, cosb, sinb, bqk, out, dbg)

    nc.compile()
    return nc


def _body(tc, xt, wq, wk, wv, wo, cosb, sinb, bqk, out, dbg={}):
    nc = tc.nc
    from contextlib import ExitStack
    with ExitStack() as ctx:
        consts = ctx.enter_context(tc.tile_pool(name="consts", bufs=1))
        work = ctx.enter_context(tc.tile_pool(name="work", bufs=2))
        work3 = ctx.enter_context(tc.tile_pool(name="work3", bufs=4))
        # single 3-deep [128,1024] ring (6 banks) shared by scores, proj
        # chains and out-proj tiles + 2 banks for the ctx accumulators
        ps_big = ctx.enter_context(
            tc.tile_pool(name="psb", bufs=3, space="PSUM"))
        ps_cx = ctx.enter_context(
            tc.tile_pool(name="psc", bufs=2, space="PSUM"))

        # ---- constants / persistent SBUF ----
        # DMA order matters: the first Q projection chain needs wq + the
        # first 512-col block of every xt chunk, so those go first and the
        # PE can start ~4us in instead of waiting for the full load.
        # few big DMAs: per-DMA issue overhead (~0.6us) dominates the
        # startup when the first chain waits on many small transfers
        wq_sb = consts.tile([128, CC * DL], BF16)
        # xt per (tc, ci-half): [128, 4*512] tiles
        xt_tc = [consts.tile([128, 4 * 512], BF16, name=f"xtt{tcn}_{h}")
                 for tcn in range(NT) for h in range(2)]

        def xt_ap(ci, tcn, lo, width):
            return xt_tc[tcn * 2 + ci // 4][
                :, (ci % 4) * 512 + lo:(ci % 4) * 512 + lo + width]

        def load_xt(tcn, nsplit=1):
            ts0 = tcn * 512
            for h in range(2):
                for j in range(nsplit):
                    w = 512 // nsplit
                    nc.sync.dma_start(
                        xt_tc[tcn * 2 + h][:, :].rearrange(
                            "p (cc t) -> p cc t", cc=4)[:, :, j * w:(j + 1) * w],
                        xt[h * 512:(h + 1) * 512,
                           ts0 + j * w:ts0 + (j + 1) * w].rearrange(
                            "(cc p) t -> p cc t", p=128))

        for h in range(2):
            nc.sync.dma_start(
                wq_sb[:, h * 4 * DL:(h + 1) * 4 * DL].rearrange(
                    "p (cc m) -> p cc m", cc=4),
                wq[h * 512:(h + 1) * 512, :].rearrange(
                    "(cc p) m -> p cc m", p=128))
        load_xt(0)
        wk_sb = consts.tile([128, CC * DL], BF16)
        nc.sync.dma_start(
            wk_sb[:, :].rearrange("p (cc m) -> p cc m", cc=CC),
            wk.rearrange("(cc p) m -> p cc m", p=128))
        bqk_sb = consts.tile([128, 4], F32)
        nc.sync.dma_start(bqk_sb[:, :], bqk)
        cos_sb = consts.tile([128, T], BF16)
        nc.sync.dma_start(cos_sb[:, :], cosb)
        sin_sb = consts.tile([128, T], BF16)
        nc.sync.dma_start(sin_sb[:, :], sinb)
        for tcn in range(1, NT):
            load_xt(tcn)
        wv_sb = consts.tile([128, CC * DL], BF16)
        nc.sync.dma_start(
            wv_sb[:, :].rearrange("p (cc m) -> p cc m", cc=CC),
            wv.rearrange("(cc p) m -> p cc m", p=128))
        wo_sb = consts.tile([128, NP * D], BF16)
        nc.sync.dma_start(
            wo_sb[:, :].rearrange("p (pr d) -> p pr d", pr=NP),
            wo.rearrange("(pr p) d -> p pr d", p=128))

        qrot = [consts.tile([128, T], BF16, name=f"qrot{p}")
                for p in range(NP)]
        krot = [consts.tile([128, T], BF16, name=f"krot{p}")
                for p in range(NP)]
        # V tiles: per k-tile, HL blocks of (64 v-dims + ones column)
        vh = consts.tile([128, NKT * HL * (HD + 1)], BF16)
        nc.gpsimd.memset(
            vh[:, :].rearrange("p (g c) -> p g c", c=HD + 1)[:, :, HD:HD + 1],
            1.0)

        # ======== Q/K projections + RoPE ========
        def qk_chain(name, pair, tcn):
            wsb = wq_sb if name == "q" else wk_sb
            bcol = 0 if name == "q" else 2
            dst = qrot[pair] if name == "q" else krot[pair]
            ts = slice(tcn * 512, (tcn + 1) * 512)
            pp = ps_big.tile([128, 1024], F32, tag="sp",
                            name=f"pp_{name}_{pair}_{tcn}")
            for ci in range(CC):
                nc.tensor.matmul(
                    pp[:, 0:512],
                    wsb[:, ci * DL + pair * 128:ci * DL + (pair + 1) * 128],
                    xt_ap(ci, tcn, 0, 512),
                    start=(ci == 0), stop=(ci == CC - 1))
            xb = work.tile([128, 512], BF16, tag="xb")
            nc.scalar.activation(
                xb[:, :], pp[:, 0:512],
                mybir.ActivationFunctionType.Identity,
                bias=bqk_sb[:, bcol + pair:bcol + pair + 1], scale=1.0)
            sh = work.tile([128, 512], BF16, tag="sh")
            nc.vector.stream_shuffle(sh[:, :], xb[:, :], SWAP_MASK)
            m1 = work.tile([128, 512], BF16, tag="m1")
            nc.vector.tensor_mul(m1[:, :], xb[:, :], cos_sb[:, ts])
            m2 = work.tile([128, 512], BF16, tag="m2")
            nc.vector.tensor_mul(m2[:, :], sh[:, :], sin_sb[:, ts])
            nc.vector.tensor_add(dst[:, ts], m1[:, :], m2[:, :])

        # tc-outer so each new 512-col block of xt serves 4 chains before
        # the next block is needed (keeps ahead of the DMA stream)
        for tcn in range(NT):
            for name in ("q", "k"):
                for pair in range(NP):
                    qk_chain(name, pair, tcn)

        # ======== V natural projection ========
        for tt in range(NKT):
            vp = ps_big.tile([128, 1024], F32, tag="sp", name=f"vp_{tt}")
            to = (tt % 4) * 128
            for ci in range(CC):
                nc.tensor.matmul(
                    vp[:, 0:DL],
                    xt_ap(ci, tt // 4, to, 128),
                    wv_sb[:, ci * DL:(ci + 1) * DL],
                    start=(ci == 0), stop=(ci == CC - 1))
            dst3 = vh[:, tt * HL * (HD + 1):(tt + 1) * HL * (HD + 1)]
            nc.scalar.activation(
                dst3.rearrange("p (h c) -> p h c", c=HD + 1)[:, :, 0:HD],
                vp[:, 0:DL].rearrange("p (h c) -> p h c", c=HD),
                mybir.ActivationFunctionType.Copy)

        if "dbg_qrot" in dbg:
            nc.sync.dma_start(dbg["dbg_qrot"], qrot[0][:, :])
        if "dbg_krot" in dbg:
            nc.sync.dma_start(dbg["dbg_krot"], krot[0][:, :])
        if "dbg_vh" in dbg:
            nc.sync.dma_start(dbg["dbg_vh"], vh[:, :])

        # ======== attention (per 512-wide q chunk) + out projection ====
        # The out projection of chunk qc-1 is software-pipelined into the
        # kt loop of chunk qc so the in-order PE never stalls waiting for
        # the normalize chain (reciprocal -> broadcast -> mul -> dma).
        stk_tiles = [[consts.tile([128, 512], BF16, name=f"stk{i}_{p}")
                      for p in range(NP)] for i in range(2)]

        def out_proj_tsub(qc, tsub, drain=False):
            # op lives in the proj-phase "pp" ring (idle during attention)
            # so score matmuls never stall behind osb copies.
            stk = stk_tiles[qc % 2]
            row0 = qc * 512 + tsub * 128
            osb = work.tile([128, D], BF16, tag="osb")
            op = ps_big.tile([128, 1024], F32, tag="sp",
                             name=f"op_{qc}_{tsub}")
            for dc in range(2):
                for pair in range(NP):
                    nc.tensor.matmul(
                        op[:, dc * 512:(dc + 1) * 512],
                        stk[pair][:, tsub * 128:(tsub + 1) * 128],
                        wo_sb[:, pair * D + dc * 512:
                              pair * D + (dc + 1) * 512],
                        start=(pair == 0), stop=(pair == NP - 1))
            if drain:
                nc.scalar.copy(osb[:, 0:512], op[:, 0:512])
                nc.vector.tensor_copy(osb[:, 512:1024], op[:, 512:1024])
            else:
                nc.vector.tensor_copy(osb[:, :], op[:, :])
            nc.sync.dma_start(out[row0:row0 + 128, :], osb[:, :])

        for qc in range(NT):
            qs = slice(qc * 512, (qc + 1) * 512)
            stk = stk_tiles[qc % 2]
            for pair in range(NP):
                cx = [ps_cx.tile([HD + 1, 512], F32, tag="cx",
                                 name=f"cx_{qc}_{pair}_{h}")
                      for h in range(2)]
                pts = {}

                def emit_pv(kt):
                    for h in range(2):
                        g = (pair * 2 + h)
                        base = kt * HL * (HD + 1) + g * (HD + 1)
                        nc.tensor.matmul(
                            cx[h][:, :],
                            vh[:, base:base + HD + 1],
                            pts[kt][:, h * 512:(h + 1) * 512],
                            start=(kt == 0), stop=(kt == NKT - 1))
                    del pts[kt]

                for kt in range(NKT):
                    sp = ps_big.tile([128, 1024], F32, tag="sp",
                                     name=f"sp_{qc}_{pair}_{kt}")
                    for h in range(2):
                        hs = slice(h * HD, (h + 1) * HD)
                        nc.tensor.matmul(
                            sp[:, h * 512:(h + 1) * 512],
                            krot[pair][hs, kt * 128:(kt + 1) * 128],
                            qrot[pair][hs, qs], start=True, stop=True)
                    pt = work3.tile([128, 1024], BF16, tag="pt")
                    pts[kt] = pt
                    nc.scalar.activation(
                        pt[:, :], sp[:, :],
                        mybir.ActivationFunctionType.Exp, scale=0.125)
                    if qc == 0 and pair == 0 and kt == 0 and "dbg_pt" in dbg:
                        tpt = work.tile([128, 1024], BF16, tag="tpt", bufs=1)
                        nc.vector.tensor_copy(tpt[:, :], pt[:, :])
                        nc.sync.dma_start(dbg["dbg_pt"], tpt[:, :])
                    # PV runs two kt behind S: covers both the exp latency
                    # (pt[kt] ready ~1.4us after S[kt]) and the first PV's
                    # wait on the cx ring (previous pair's normalize)
                    if kt >= 2:
                        emit_pv(kt - 2)
                    # pipelined out projection of the previous q chunk
                    # (2 tsubs in each pair's kt loop, spread out)
                    if qc > 0 and kt % 8 == 5:
                        out_proj_tsub(qc - 1, pair * 2 + kt // 8)
                emit_pv(NKT - 2)
                emit_pv(NKT - 1)

                if qc == 0 and pair == 0 and "dbg_cx" in dbg:
                    tcx = work.tile([HD + 1, 512], F32, tag="tcx", bufs=1)
                    nc.vector.tensor_copy(tcx[:, :], cx[0][:, :])
                    nc.sync.dma_start(dbg["dbg_cx"], tcx[:, :])
                # Drain cx PSUM fast with plain copies (frees the ring for
                # the next pair), then normalize from the SBUF copies.  On
                # the final pair nothing reuses the ring, so normalize
                # straight from PSUM (one hop less before the drain).
                last = (qc == NT - 1)
                stk_p = stk[pair]
                ctxsb = []
                den64s = []
                for h in range(2):
                    if last:
                        ctxsb.append(None)
                    else:
                        csb = work.tile([HD, 512], F32, tag=f"ctxsb{h}")
                        nc.vector.tensor_copy(csb[:, :], cx[h][0:HD, :])
                        ctxsb.append(csb)
                    den64 = work.tile([HD + 1, 512], F32, tag="den64")
                    nc.vector.tensor_copy(den64[HD:HD + 1, :],
                                          cx[h][HD:HD + 1, :])
                    den64s.append(den64)
                for h in range(2):
                    den0 = work.tile([1, 512], F32, tag="den0")
                    nc.sync.dma_start(den0[0:1, :], den64s[h][HD:HD + 1, :])
                    rcp0 = work.tile([1, 512], F32, tag="rcp0")
                    nc.vector.reciprocal_approx_fast(rcp0[0:1, :],
                                                     den0[0:1, :])
                    bc = work.tile([HD, 512], F32, tag="bc")
                    nc.gpsimd.partition_broadcast(bc[:, :], rcp0[0:1, :],
                                                  channels=HD)
                    src = cx[h][0:HD, :] if last else ctxsb[h][:, :]
                    if h == 0:
                        nc.vector.tensor_mul(stk_p[0:HD, :], src, bc[:, :])
                    else:
                        cn1 = work.tile([HD, 512], BF16, tag="cn1")
                        nc.vector.tensor_mul(cn1[:, :], src, bc[:, :])
                        nc.sync.dma_start(stk_p[HD:128, :], cn1[:, :])

            if qc == 0 and "dbg_stk" in dbg:
                nc.sync.dma_start(dbg["dbg_stk"], stk[0][:, :])

        # drain: out projection of the final q chunk.  The pair0 chain
        # pieces depend only on stk[pair0] (ready since mid-chunk), so
        # open as many accumulation groups as the ring allows before the
        # pair1 normalize lands.
        qcl = NT - 1
        stk = stk_tiles[qcl % 2]

        def drain_piece(op, tsub, pair):
            for dc in range(2):
                nc.tensor.matmul(
                    op[:, dc * 512:(dc + 1) * 512],
                    stk[pair][:, tsub * 128:(tsub + 1) * 128],
                    wo_sb[:, pair * D + dc * 512:pair * D + (dc + 1) * 512],
                    start=(pair == 0), stop=(pair == NP - 1))

        def drain_close(op, tsub):
            drain_piece(op, tsub, 1)
            row0 = qcl * 512 + tsub * 128
            osb = work.tile([128, D], BF16, tag="osb")
            nc.scalar.copy(osb[:, 0:512], op[:, 0:512])
            nc.vector.tensor_copy(osb[:, 512:1024], op[:, 512:1024])
            nc.sync.dma_start(out[row0:row0 + 128, :], osb[:, :])

        dops = [ps_big.tile([128, 1024], F32, tag="sp", name=f"dop_{t}")
                for t in range(3)]
        for t in range(3):
            drain_piece(dops[t], t, 0)
        drain_close(dops[0], 0)
        op3 = ps_big.tile([128, 1024], F32, tag="sp", name="dop_3")
        drain_piece(op3, 3, 0)
        drain_close(dops[1], 1)
        drain_close(dops[2], 2)
        drain_close(op3, 3)


def _rope_tables():
    """cos/sin tables in the interleaved-pair partition layout (bf16)."""
    pos = np.arange(T, dtype=np.float32)[:, None]                 # [T, 1]
    freq_seq = np.arange(HD // 2, dtype=np.float32)
    inv_freq = (1.0 / (10000.0 ** (freq_seq / np.float32(HD // 2)))).astype(
        np.float32)
    ang = pos * inv_freq[None, :]                                 # [T, 32]
    sin = np.sin(ang).astype(np.float32)                          # [T, 32]
    cos = np.cos(ang).astype(np.float32)
    cosb = np.empty((128, T), dtype=np.float32)
    sinb = np.empty((128, T), dtype=np.float32)
    for p in range(128):
        r = p % HD
        j = r // 2
        second = r % 2
        cosb[p] = cos[:, j]
        sinb[p] = sin[:, j] if second else -sin[:, j]
    return cosb.astype(bfloat16), sinb.astype(bfloat16)


def _perm():
    """interleaved-pair permutation of each head's 64 dims:
    new[h*64 + 2j] = old[h*64 + j]; new[h*64 + 2j + 1] = old[h*64 + 32 + j]"""
    p = np.arange(DL)
    return (p // HD) * HD + (p % HD) // 2 + (p % 2) * (HD // 2)


def kernel(**inputs):
    global LAST_RESULT
    x = np.asarray(inputs["x"], dtype=np.float32)
    Wq = np.asarray(inputs["Wq"], dtype=np.float32)
    Wk = np.asarray(inputs["Wk"], dtype=np.float32)
    Wv = np.asarray(inputs["Wv"], dtype=np.float32)
    Wo = np.asarray(inputs["Wo"], dtype=np.float32)
    bq = np.asarray(inputs["bq"], dtype=np.float32)
    bk = np.asarray(inputs["bk"], dtype=np.float32)
    bv = np.asarray(inputs["bv"], dtype=np.float32)
    bo = np.asarray(inputs["bo"], dtype=np.float32)

    if "nc" not in _CACHE:
        _CACHE["nc"] = _build_nc()
    nc = _CACHE["nc"]

    xT = np.ascontiguousarray(x.transpose(0, 2, 1)).astype(bfloat16)
    cosb, sinb = _rope_tables()
    perm = _perm()

    in_maps = []
    for c in range(N_CORES):
        b, hg = c // 4, c % 4
        cs = slice(hg * DL, (hg + 1) * DL)
        bq_c = bq[cs][perm]
        bk_c = bk[cs][perm]
        bqk_c = np.stack([bq_c[0:128], bq_c[128:256],
                          bk_c[0:128], bk_c[128:256]], axis=1)
        in_maps.append({
            "xt": xT[b],
            "wq": np.ascontiguousarray(Wq[:, cs][:, perm]).astype(bfloat16),
            "wk": np.ascontiguousarray(Wk[:, cs][:, perm]).astype(bfloat16),
            "wv": np.ascontiguousarray(Wv[:, cs]).astype(bfloat16),
            "wo": np.ascontiguousarray(Wo[cs, :]).astype(bfloat16),
            "cosb": cosb, "sinb": sinb,
            "bqk": np.ascontiguousarray(bqk_c).astype(np.float32),
        })

    trace = bool(int(os.environ.get("BASS_KERNEL_TRACE", "0")))
    for attempt in range(3):
        res = run_bass_kernel_spmd(nc, in_maps,
                                   core_ids=list(range(N_CORES)),
                                   trace=trace)
        LAST_RESULT = res
        acc = np.zeros((B, T, D), dtype=np.float32)
        for c in range(N_CORES):
            acc[c // 4] += res.results[c]["out"].astype(np.float32)
        if np.isfinite(acc).all():
            break
    const_row = (bv.astype(np.float64) @ Wo.astype(np.float64)
                 + bo.astype(np.float64)).astype(np.float32)
    return acc + const_row[None, None, :]
